# revision 6
# baseline (speedup 1.0000x reference)
"""Trainium2 Bass kernel for nn_MegaMerge.

Computes G = concat([h0^T, c2q, h0^T*c2q, h0^T*q2c], axis=0) where
h: [1, T, D] f32, c2q/q2c: [D, T] f32, output G: [4D, T] f32
with T=4096, D=2048.

Sharding: T (context length) split contiguously across 8 NeuronCores
(512 columns each). Fully elementwise per position -> no communication.
Host-side numpy does the shard/unshard so every device-side DMA is
contiguous. Per core:
  - load h shard [512, 2048] in natural (t-major) layout, 4x 1MiB DMAs
  - TensorE transpose (fp32 matmul-with-identity) 128x128 tiles -> PSUM
  - ScalarE copies PSUM -> SBUF (builds h0^T tiles [128, 4*512])
  - VectorE elementwise muls for the two product blocks
  - contiguous 1MiB DMA stores of the four output blocks
"""

import numpy as np

import concourse.bass as bass
import concourse.bacc as bacc
import concourse.mybir as mybir
from concourse.tile import TileContext
from concourse.masks import make_identity
from concourse.bass_utils import run_bass_kernel_spmd

N_CORES = 8
T = 4096
D = 2048
TS = T // N_CORES  # 512: per-core shard of the T axis
P = 128
R = D // P         # 16 partition tiles along D
GRP = 4            # r-tiles fused per group (free dim 4*512 = 2048)
NG = R // GRP      # 4 groups
A = TS // P        # 4 t-tiles of the natural-layout h shard

F32 = mybir.dt.float32


def build_nc() -> bass.Bass:
    # Bacc (not plain Bass): its finalize() runs the wait-splitting
    # passes (move_matmul_waits_to_ldweights, generate_event_semaphores)
    # that the walrus TRN2 codegen requires for Tile-generated matmuls.
    nc = bacc.Bacc()
    h = nc.dram_tensor("h", [TS, D], F32, kind="ExternalInput")
    c2q = nc.dram_tensor("c2q", [D, TS], F32, kind="ExternalInput")
    q2c = nc.dram_tensor("q2c", [D, TS], F32, kind="ExternalInput")
    g = nc.dram_tensor("g", [4 * D, TS], F32, kind="ExternalOutput")

    # row index of c2q/q2c/g-block = grp*512 + c*128 + p.
    # DRAM views are permuted to [p, c, t] so the SBUF side of every DMA
    # stays partition-major (SBUF APs must have the partition dim first).
    c2q_v = c2q.rearrange("(gr c p) t -> gr p c t", c=GRP, p=P)
    q2c_v = q2c.rearrange("(gr c p) t -> gr p c t", c=GRP, p=P)
    g_v = g.rearrange("(b gr c p) t -> b gr p c t", gr=NG, c=GRP, p=P)

    with TileContext(nc) as tc:
        with (
            tc.tile_pool(name="const", bufs=1) as cpool,
            tc.tile_pool(name="hpool", bufs=1) as hpool,
            tc.tile_pool(name="work", bufs=3) as wpool,
            tc.tile_pool(name="pspool", bufs=8, space="PSUM") as ppool,
        ):
            ident = cpool.tile([P, P], F32)
            make_identity(nc, ident[:])

            # resident natural-layout h shard: 4 tiles [128 t, 2048 d]
            h_nat = []
            for a in range(A):
                ht = hpool.tile([P, D], F32, tag=f"hnat{a}")
                nc.sync.dma_start(out=ht[:], in_=h[a * P:(a + 1) * P, :])
                h_nat.append(ht)

            for gi in range(NG):
                h0t = wpool.tile([P, GRP, TS], F32, tag="h0t")
                for c in range(GRP):
                    r = gi * GRP + c
                    for a in range(A):
                        ps = ppool.tile([P, P], F32, tag="tps")
                        nc.tensor.transpose(
                            ps[:], h_nat[a][:, r * P:(r + 1) * P], ident[:]
                        )
                        nc.scalar.copy(
                            out=h0t[:, c, a * P:(a + 1) * P], in_=ps[:]
                        )
                nc.sync.dma_start(
                    out=g_v[0, gi], in_=h0t[:]
                )

                cq = wpool.tile([P, GRP, TS], F32, tag="cq")
                nc.sync.dma_start(
                    out=cq[:], in_=c2q_v[gi]
                )
                nc.sync.dma_start(
                    out=g_v[1, gi], in_=cq[:]
                )

                p1 = wpool.tile([P, GRP, TS], F32, tag="p1")
                nc.vector.tensor_mul(out=p1[:], in0=h0t[:], in1=cq[:])
                nc.sync.dma_start(
                    out=g_v[2, gi], in_=p1[:]
                )

                qc = wpool.tile([P, GRP, TS], F32, tag="qc")
                nc.sync.dma_start(
                    out=qc[:], in_=q2c_v[gi]
                )
                p2 = wpool.tile([P, GRP, TS], F32, tag="p2")
                nc.vector.tensor_mul(out=p2[:], in0=h0t[:], in1=qc[:])
                nc.sync.dma_start(
                    out=g_v[3, gi], in_=p2[:]
                )
    nc.finalize()
    return nc


_NC_CACHE: dict = {}


def _get_nc() -> bass.Bass:
    if "nc" not in _NC_CACHE:
        _NC_CACHE["nc"] = build_nc()
    return _NC_CACHE["nc"]


def make_in_maps(h, c2q, q2c):
    h = np.asarray(h)
    c2q = np.asarray(c2q)
    q2c = np.asarray(q2c)
    h0 = np.ascontiguousarray(h.reshape(T, D).astype(np.float32, copy=False))
    in_maps = []
    for m in range(N_CORES):
        sl = slice(m * TS, (m + 1) * TS)
        in_maps.append(
            {
                "h": np.ascontiguousarray(h0[sl, :]),
                "c2q": np.ascontiguousarray(
                    c2q[:, sl].astype(np.float32, copy=False)
                ),
                "q2c": np.ascontiguousarray(
                    q2c[:, sl].astype(np.float32, copy=False)
                ),
            }
        )
    return in_maps


def gather_out(results) -> np.ndarray:
    return np.concatenate(
        [results[m]["g"] for m in range(N_CORES)], axis=1
    )


def kernel(h, c2q, q2c, max_context_length=None, **_unused) -> np.ndarray:
    in_maps = make_in_maps(h, c2q, q2c)
    res = run_bass_kernel_spmd(_get_nc(), in_maps, list(range(N_CORES)))
    return gather_out(res.results)


# revision 7
# speedup vs baseline: 1.0558x; 1.0558x over previous
"""Trainium2 Bass kernel for nn_MegaMerge.

Computes G = concat([h0^T, c2q, h0^T*c2q, h0^T*q2c], axis=0) where
h: [1, T, D] f32, c2q/q2c: [D, T] f32, output G: [4D, T] f32
with T=4096, D=2048.

Sharding: T (context length) split contiguously across 8 NeuronCores
(512 columns each). Fully elementwise per position -> no communication.

Device-side layout trick: the host pre-permutes c2q/q2c shards (and
un-permutes the output) into [group, partition, c, t] order so that
EVERY device DMA is a fully contiguous ~1 MiB transfer with 8 KiB per
partition. Per core:
  - load h shard [512, 2048] natural (t-major), 4x 1MiB contiguous DMAs
  - all input loads issued up front (no store waits ahead of them in
    the HWDGE FIFO)
  - TensorE transpose (fp32 matmul-with-identity), 4x 128x128 tiles
    into one [128, 512] PSUM bank
  - ScalarE copies PSUM bank -> SBUF (builds h0^T tiles [128, 4, 512])
  - VectorE elementwise muls for the two product blocks
  - contiguous 1MiB DMA stores of the four output blocks per group
"""

import numpy as np

import concourse.bass as bass
import concourse.bacc as bacc
import concourse.mybir as mybir
from concourse.tile import TileContext
from concourse.masks import make_identity
from concourse.bass_utils import run_bass_kernel_spmd

N_CORES = 8
T = 4096
D = 2048
TS = T // N_CORES  # 512: per-core shard of the T axis
P = 128
R = D // P         # 16 partition tiles along D
GRP = 4            # r-tiles fused per group (free dim 4*512 = 2048)
NG = R // GRP      # 4 groups
A = TS // P        # 4 t-tiles of the natural-layout h shard

F32 = mybir.dt.float32


def build_nc() -> bass.Bass:
    # Bacc (not plain Bass): its finalize() runs the wait-splitting
    # passes (move_matmul_waits_to_ldweights, generate_event_semaphores)
    # that the walrus TRN2 codegen requires for Tile-generated matmuls.
    nc = bacc.Bacc()
    h = nc.dram_tensor("h", [TS, D], F32, kind="ExternalInput")
    # pre-permuted on host: [gi, p, c, t]; row gi*512 + c*128 + p of the
    # logical [D, TS] shard lands at [gi, p, c, :]
    c2q = nc.dram_tensor("c2q", [NG, P, GRP, TS], F32, kind="ExternalInput")
    q2c = nc.dram_tensor("q2c", [NG, P, GRP, TS], F32, kind="ExternalInput")
    # output, same permuted layout plus leading block dim
    g = nc.dram_tensor("g", [4, NG, P, GRP, TS], F32, kind="ExternalOutput")

    with TileContext(nc) as tc:
        with (
            tc.tile_pool(name="const", bufs=1) as cpool,
            tc.tile_pool(name="hpool", bufs=1) as hpool,
            tc.tile_pool(name="inpool", bufs=NG) as inpool,
            tc.tile_pool(name="work", bufs=3) as wpool,
            tc.tile_pool(name="pspool", bufs=8, space="PSUM") as ppool,
        ):
            ident = cpool.tile([P, P], F32)
            make_identity(nc, ident[:])

            # ---- all input loads issued up front (single HWDGE FIFO:
            # nothing with a data-dependent wait may precede them) ----
            h_nat = []
            for a in range(A):
                ht = hpool.tile([P, D], F32, tag=f"hnat{a}")
                nc.sync.dma_start(out=ht[:], in_=h[a * P:(a + 1) * P, :])
                h_nat.append(ht)
            cqs, qcs = [], []
            for gi in range(NG):
                cq = inpool.tile([P, GRP, TS], F32, tag="cq")
                nc.sync.dma_start(out=cq[:], in_=c2q[gi])
                qc = inpool.tile([P, GRP, TS], F32, tag="qc")
                nc.sync.dma_start(out=qc[:], in_=q2c[gi])
                cqs.append(cq)
                qcs.append(qc)

            for gi in range(NG):
                h0t = wpool.tile([P, GRP, TS], F32, tag="h0t")
                for c in range(GRP):
                    r = gi * GRP + c
                    ps = ppool.tile([P, TS], F32, tag="tps")
                    for a in range(A):
                        nc.tensor.transpose(
                            ps[:, a * P:(a + 1) * P],
                            h_nat[a][:, r * P:(r + 1) * P],
                            ident[:],
                        )
                    nc.scalar.copy(out=h0t[:, c, :], in_=ps[:])
                nc.sync.dma_start(out=g[0, gi], in_=h0t[:])
                nc.sync.dma_start(out=g[1, gi], in_=cqs[gi][:])

                p1 = wpool.tile([P, GRP, TS], F32, tag="p1")
                nc.vector.tensor_mul(out=p1[:], in0=h0t[:], in1=cqs[gi][:])
                nc.sync.dma_start(out=g[2, gi], in_=p1[:])

                p2 = wpool.tile([P, GRP, TS], F32, tag="p2")
                nc.vector.tensor_mul(out=p2[:], in0=h0t[:], in1=qcs[gi][:])
                nc.sync.dma_start(out=g[3, gi], in_=p2[:])
    nc.finalize()
    return nc


_NC_CACHE: dict = {}


def _get_nc() -> bass.Bass:
    if "nc" not in _NC_CACHE:
        _NC_CACHE["nc"] = build_nc()
    return _NC_CACHE["nc"]


def _permute_in(x_shard: np.ndarray) -> np.ndarray:
    # [D, TS] -> [NG, P, GRP, TS] with row gi*512 + c*128 + p -> [gi, p, c]
    v = x_shard.reshape(NG, GRP, P, TS).transpose(0, 2, 1, 3)
    return np.ascontiguousarray(v)


def make_in_maps(h, c2q, q2c):
    h = np.asarray(h)
    c2q = np.asarray(c2q, dtype=np.float32)
    q2c = np.asarray(q2c, dtype=np.float32)
    h0 = np.ascontiguousarray(h.reshape(T, D).astype(np.float32, copy=False))
    in_maps = []
    for m in range(N_CORES):
        sl = slice(m * TS, (m + 1) * TS)
        in_maps.append(
            {
                "h": np.ascontiguousarray(h0[sl, :]),
                "c2q": _permute_in(c2q[:, sl]),
                "q2c": _permute_in(q2c[:, sl]),
            }
        )
    return in_maps


def gather_out(results) -> np.ndarray:
    # per-core g: [4, NG, P, GRP, TS] -> [4*D, TS]; then concat over T
    outs = []
    for m in range(N_CORES):
        gm = results[m]["g"]
        outs.append(gm.transpose(0, 1, 3, 2, 4).reshape(4 * D, TS))
    return np.ascontiguousarray(np.concatenate(outs, axis=1))


def kernel(h, c2q, q2c, max_context_length=None, **_unused) -> np.ndarray:
    in_maps = make_in_maps(h, c2q, q2c)
    res = run_bass_kernel_spmd(_get_nc(), in_maps, list(range(N_CORES)))
    return gather_out(res.results)
